# revision 21
# baseline (speedup 1.0000x reference)
"""Trainium2 Bass kernel for nn_CCL__69277822485245 (spectral conv via DCT/FFT).

Math: the reference's rFFT along W cancels into a circular 5-tap convolution,
and the DCT-II sandwich M @ diag(D[:,s]) @ D collapses into 5 dense 128x128
matrices G_s (precomputed on host). Per batch element:

    u_s[i, m, w] = sum_h G_s[m, h] x[i, h, w]                  (stage 1)
    out[o, m, n] = sum_{s,t,i} W[o,i,s,t] u_s[i, m, (n-t)%W] + bias[o]   (stage 2)

Sharding: data-parallel over batch B=8 across the 8 NeuronCores (1 each).

Schedule (per core): u is computed resident for a full m-half (64 m's) over
all 132 w-columns (128 + 4 wrap columns for the circular t-shifts), so no
halo recompute. Stage-1 psum->SBUF casts split across the Vector and Scalar
engines. Stage 2 contracts (i, s-pair) with K=128 in 13 accumulation groups
per output chunk: 10 groups for s-pairs (0,1) and (2,3) x 5 taps, plus 3
groups pairing (s=4, t) with (s=4, t+1), the second tap coming from a
one-column-shifted copy of u_4 placed on partitions 64:127 by small
SBUF->SBUF DMAs. Output is written in (chunk, o, (j,m)) layout so the
stage-2 evacuation and DMA are fully contiguous; the host reassembles.

x is loaded unduplicated (2.1MB) and duplicated on-chip (DVE SBUF->SBUF
bf16 copies run in 4x mode) so the stationary spans all 128 PE columns.
"""

import numpy as np

H = 128
W = 128
CI = 64
CO = 128
KH = 5
KW = 5
B = 8

MH = 64          # m-half processed per outer iteration
HALO = 4         # wrap columns at the front of u's j axis
JW = W + HALO    # 132 j-columns; w = (j - 4) % 128
NCH = 16         # stage-2 output chunks per m-half (8 w-columns each)
LA = 4           # stage-1 tile emission lookahead beyond chunk needs

DTYPE = "bf16"

_PROG = None
_CONSTS = None
_RUN_OPTS = {}     # test harness may set e.g. {"trace": True, "trace_cores": [0]}
_LAST_RESULT = None

G10 = [(t, c) for t in range(KH) for c in range(2)]   # K=128 (i, s=2c / 2c+1)


def _np_dt():
    if DTYPE == "bf16":
        import ml_dtypes
        return ml_dtypes.bfloat16
    return np.float32


def _build_consts():
    n = np.arange(H, dtype=np.float64)
    ang = np.pi * (2.0 * n[None, :] + 1.0) * n[:, None] / (2.0 * H)  # [k, h]
    D = 2.0 * np.cos(ang)
    wgt = np.where(n == 0, 0.5, 1.0)
    M = (np.cos(ang).T * wgt[None, :]) / (2.0 * H)                    # [m, k]
    G = np.stack([M @ (D[:, s:s + 1] * D) for s in range(KH)])        # [s, m, h]
    G = G[[0, 2, 4, 1, 3]]   # s-order so each half's psum->u copy is contiguous
    # rhs layout [h, (mh, sidx, ml)]: col = mh*320 + sidx*64 + ml
    GT = (G.transpose(2, 0, 1)                # [h, s, m]
            .reshape(H, KH, 2, MH)            # [h, s, mh, ml]
            .transpose(0, 2, 1, 3)            # [h, mh, s, ml]
            .reshape(H, KH * H))
    return np.ascontiguousarray(GT).astype(_np_dt())


def _build_program():
    import concourse.mybir as mybir
    import concourse.tile as tile
    from concourse import bacc

    f32 = mybir.dt.float32
    mmdt = {"bf16": mybir.dt.bfloat16,
            "f32r": mybir.dt.float32r,
            "f32": mybir.dt.float32}[DTYPE]

    nc = bacc.Bacc("TRN2", target_bir_lowering=False, debug=False,
                   enable_asserts=False, num_devices=B)
    x_d = nc.dram_tensor("x", [H, W, CI], mmdt, kind="ExternalInput").ap()
    g_d = nc.dram_tensor("g", [H, KH * H], mmdt, kind="ExternalInput").ap()
    w_d = nc.dram_tensor("wt", [128, 13 * CO], mmdt,
                         kind="ExternalInput").ap()
    b_d = nc.dram_tensor("bias", [CO, 1], f32, kind="ExternalInput").ap()
    o_d = nc.dram_tensor("out", [2 * NCH, CO, 512], f32,
                         kind="ExternalOutput").ap()

    with tile.TileContext(nc) as tc:
        with (
            tc.tile_pool(name="const", bufs=1) as cpool,
            tc.tile_pool(name="u", bufs=1) as upool,
            tc.tile_pool(name="oacc", bufs=3) as opool,
            tc.tile_pool(name="ps1", bufs=3, space="PSUM") as ps1,
            tc.tile_pool(name="ps2", bufs=2, space="PSUM") as ps2,
        ):
            # DMA descriptors drain FIFO per engine; order by first use:
            # gt + x chunk 7 gate the first matmul, wt only the first
            # stage-2 chunk (~12 tiles in), later x chunks stream behind.
            gt = cpool.tile([H, KH * H], mmdt)
            nc.sync.dma_start(gt[:], g_d)
            xT = cpool.tile([H, W * 2 * CI], mmdt)
            x4 = xT[:].rearrange("p (w di) -> p w di", di=2 * CI)

            def load_x_span(w0, w1):
                # HBM load of the 64 real channels, then duplicate on-chip
                # (DVE SBUF->SBUF bf16 runs in 4x mode) so the stage-1
                # stationary spans all 128 PE columns.
                sl = slice(w0, w1)
                nc.sync.dma_start(x4[:, sl, 0:CI], x_d[:, sl, :])
                nc.vector.tensor_copy(x4[:, sl, CI:2 * CI], x4[:, sl, 0:CI])

            load_x_span(112, 128)
            load_x_span(0, 16)
            wt = cpool.tile([128, 13 * CO], mmdt)
            nc.sync.dma_start(wt[:], w_d)
            bt = cpool.tile([CO, 1], f32)
            nc.sync.dma_start(bt[:], b_d)
            for wc in [1, 2, 3, 4, 5, 6]:
                load_x_span(wc * 16, (wc + 1) * 16)

            import concourse.mybir as _mb

            # GPSIMD cannot read PSUM (BIR verifier) -> DVE + Act only
            cast_engines = [nc.vector.tensor_copy, nc.scalar.copy]
            state = {"cast_idx": 0}

            def next_eng():
                e = cast_engines[state["cast_idx"] % 2]
                state["cast_idx"] += 1
                return e

            def s1_tile(mh, jp, u4):
                p1 = ps1.tile([128, 1024], f32)
                for dj in range(2):
                    wg = (2 * jp + dj - HALO) % W
                    nc.tensor.matmul(p1[:, dj * 512:dj * 512 + KH * MH],
                                     x4[:, wg, :],
                                     gt[:, mh * KH * MH:(mh + 1) * KH * MH],
                                     start=True, stop=True)
                pv = p1[:].rearrange("p (j s m) -> p j s m", j=2, s=8)
                # psum s-order [0,2,4,1,3]: half0 cols 0:192, half1 192:320;
                # alternate which engine gets the bigger lo cast per tile.
                next_eng()(
                    u4[0:64, :, 2 * jp:2 * jp + 2, :].transpose([0, 2, 1, 3]),
                    pv[0:64, :, 0:3, :])
                next_eng()(
                    u4[64:128, 0:2, 2 * jp:2 * jp + 2, :].transpose([0, 2, 1, 3]),
                    pv[64:128, :, 3:5, :])
                state["cast_idx"] += 1

            def s4_shift_piece(p, u4):
                # us4sh[j] = u_s4[j-1] on partitions 64:127 (c=2 slot)
                j0, j1 = (1, 8) if p == 0 else (8 * p, 8 * p + 8)
                j1 = min(j1, JW)
                nc.sync.dma_start(u4[64:128, 2, j0:j1, :],
                                  u4[0:64, 2, j0 - 1:j1 - 1, :])

            def s2_chunk(mh, ch, u4):
                p2 = ps2.tile([128, 512], f32)
                for gi, (t, c) in enumerate(G10):
                    rhs = u4[0:128, c, ch * 8 + HALO - t:ch * 8 + HALO - t + 8, :]
                    nc.tensor.matmul(p2[:], wt[:, gi * CO:(gi + 1) * CO], rhs,
                                     start=(gi == 0), stop=False)
                for p in range(3):
                    tp = 2 * p
                    rhs = u4[0:128, 2,
                             ch * 8 + HALO - tp:ch * 8 + HALO - tp + 8, :]
                    nc.tensor.matmul(p2[:], wt[:, (10 + p) * CO:(11 + p) * CO],
                                     rhs, start=False, stop=(p == 2))
                # evacuate in halves on both engines (DVE adds the
                # per-partition bias via tensor_scalar_add)
                oa = opool.tile([CO, 512], f32)
                nc.scalar.activation(oa[:, 0:256], p2[:, 0:256],
                                     _mb.ActivationFunctionType.Identity,
                                     bias=bt[:])
                nc.vector.tensor_scalar_add(oa[:, 256:512], p2[:, 256:512],
                                            bt[:])
                nc.sync.dma_start(o_d[mh * NCH + ch], oa[:])

            NT = JW // 2          # 66 stage-1 tiles (2 w-columns each) per mh
            NPIECE = NCH + 1      # 17 shift pieces per mh
            u4s = {}
            pieces_done = {0: 0, 1: 0}

            def emit_tile(gti):
                mh, jp = divmod(gti, NT)
                if mh not in u4s:
                    u = upool.tile([128, 3 * JW * MH], mmdt, tag=f"u{mh}")
                    u4s[mh] = u[:].rearrange("p (c j m) -> p c j m",
                                             c=3, j=JW)
                    nc.vector.memset(u4s[mh][64:128, 2, 0:1, :], 0.0)
                s1_tile(mh, jp, u4s[mh])
                # shift piece p is ready once tiles jp<=4p+3 are cast
                while pieces_done[mh] < NPIECE and \
                        jp + 1 >= min(4 * pieces_done[mh] + 4, NT):
                    s4_shift_piece(pieces_done[mh], u4s[mh])
                    pieces_done[mh] += 1

            ti = 0
            for gch in range(2 * NCH):
                mh, k = divmod(gch, NCH)
                target = min(mh * NT + min(NT, 4 * k + 8) + LA, 2 * NT)
                while ti < target:
                    emit_tile(ti)
                    ti += 1
                s2_chunk(mh, k, u4s[mh])
            while ti < 2 * NT:
                emit_tile(ti)
                ti += 1
    nc.compile()
    return nc


def _get_prog():
    global _PROG
    if _PROG is None:
        _PROG = _build_program()
    return _PROG


def _build_wstack(weight):
    # wst[(d,i), (g, o)]: 13 groups; g<10: rows 0:64 s=2c, 64:128 s=2c+1;
    # g=10+p: rows 0:64 (s=4, t=2p), rows 64:128 (s=4, t=2p+1) (zeros p=2)
    wst = np.zeros((128, 13 * CO), np.float32)
    for gi, (t, c) in enumerate(G10):
        col = gi * CO
        wst[0:64, col:col + CO] = weight[:, :, 2 * c, t].T
        wst[64:128, col:col + CO] = weight[:, :, 2 * c + 1, t].T
    for p in range(3):
        col = (10 + p) * CO
        tp = 2 * p
        wst[0:64, col:col + CO] = weight[:, :, 4, tp].T
        if tp + 1 < KW:
            wst[64:128, col:col + CO] = weight[:, :, 4, tp + 1].T
    return np.ascontiguousarray(wst).astype(_np_dt())


def kernel(x, weight, bias):
    from concourse.bass_utils import run_bass_kernel_spmd

    global _CONSTS
    if _CONSTS is None:
        _CONSTS = _build_consts()
    GT = _CONSTS

    x = np.ascontiguousarray(np.asarray(x, dtype=np.float32))
    weight = np.ascontiguousarray(np.asarray(weight, dtype=np.float32))
    bias = np.ascontiguousarray(np.asarray(bias, dtype=np.float32))

    wst = _build_wstack(weight)
    b2 = np.ascontiguousarray(bias.reshape(CO, 1))

    in_maps = []
    for b in range(B):
        xh = np.ascontiguousarray(x[b].transpose(1, 2, 0))   # [h, w, i]
        xh = xh.reshape(H, W * CI).astype(_np_dt())
        in_maps.append({"x": xh, "g": GT, "wt": wst, "bias": b2})

    res = run_bass_kernel_spmd(_get_prog(), in_maps, core_ids=list(range(B)),
                               **_RUN_OPTS)
    global _LAST_RESULT
    _LAST_RESULT = res
    outs = []
    for b in range(B):
        arr = res.results[b]["out"].reshape(2, NCH, CO, 8, MH)
        outs.append(arr.transpose(2, 0, 4, 1, 3).reshape(CO, H, W))
    out = np.stack(outs, axis=0)
    return np.ascontiguousarray(out.astype(np.float32))


# revision 23
# speedup vs baseline: 1.0428x; 1.0428x over previous
"""Trainium2 Bass kernel for nn_CCL__69277822485245 (spectral conv via DCT/FFT).

Math: the reference's rFFT along W cancels into a circular 5-tap convolution,
and the DCT-II sandwich M @ diag(D[:,s]) @ D collapses into 5 dense 128x128
matrices G_s (precomputed on host). Per batch element:

    u_s[i, m, w] = sum_h G_s[m, h] x[i, h, w]                  (stage 1)
    out[o, m, n] = sum_{s,t,i} W[o,i,s,t] u_s[i, m, (n-t)%W] + bias[o]   (stage 2)

Sharding: data-parallel over batch B=8 across the 8 NeuronCores (1 each).

Schedule (per core): u is computed resident for a full m-half (64 m's) over
all 132 w-columns (128 + 4 wrap columns for the circular t-shifts), so no
halo recompute. Stage-1 psum->SBUF casts split across the Vector and Scalar
engines. Stage 2 contracts (i, s-pair) with K=128 in 13 accumulation groups
per output chunk: 10 groups for s-pairs (0,1) and (2,3) x 5 taps, plus 3
groups pairing (s=4, t) with (s=4, t+1), the second tap coming from a
one-column-shifted copy of u_4 placed on partitions 64:127 by small
SBUF->SBUF DMAs. Output is written in (chunk, o, (j,m)) layout so the
stage-2 evacuation and DMA are fully contiguous; the host reassembles.

x is loaded unduplicated (2.1MB) and duplicated on-chip (DVE SBUF->SBUF
bf16 copies run in 4x mode) so the stationary spans all 128 PE columns.
"""

import numpy as np

H = 128
W = 128
CI = 64
CO = 128
KH = 5
KW = 5
B = 8

MH = 64          # m-half processed per outer iteration
HALO = 4         # wrap columns at the front of u's j axis
JW = W + HALO    # 132 j-columns; w = (j - 4) % 128
NCH = 16         # stage-2 output chunks per m-half (8 w-columns each)
LA = 4           # stage-1 tile emission lookahead beyond chunk needs

DTYPE = "bf16"

_PROG = None
_CONSTS = None
_RUN_OPTS = {}     # test harness may set e.g. {"trace": True, "trace_cores": [0]}
_LAST_RESULT = None

G10 = [(t, c) for t in range(KH) for c in range(2)]   # K=128 (i, s=2c / 2c+1)


def _np_dt():
    if DTYPE == "bf16":
        import ml_dtypes
        return ml_dtypes.bfloat16
    return np.float32


def _build_consts():
    n = np.arange(H, dtype=np.float64)
    ang = np.pi * (2.0 * n[None, :] + 1.0) * n[:, None] / (2.0 * H)  # [k, h]
    D = 2.0 * np.cos(ang)
    wgt = np.where(n == 0, 0.5, 1.0)
    M = (np.cos(ang).T * wgt[None, :]) / (2.0 * H)                    # [m, k]
    G = np.stack([M @ (D[:, s:s + 1] * D) for s in range(KH)])        # [s, m, h]
    G = G[[0, 2, 4, 1, 3]]   # s-order so each half's psum->u copy is contiguous
    # rhs layout [h, (mh, sidx, ml)]: col = mh*320 + sidx*64 + ml
    GT = (G.transpose(2, 0, 1)                # [h, s, m]
            .reshape(H, KH, 2, MH)            # [h, s, mh, ml]
            .transpose(0, 2, 1, 3)            # [h, mh, s, ml]
            .reshape(H, KH * H))
    return np.ascontiguousarray(GT).astype(_np_dt())


def _build_program():
    import concourse.mybir as mybir
    import concourse.tile as tile
    from concourse import bacc

    f32 = mybir.dt.float32
    mmdt = {"bf16": mybir.dt.bfloat16,
            "f32r": mybir.dt.float32r,
            "f32": mybir.dt.float32}[DTYPE]

    nc = bacc.Bacc("TRN2", target_bir_lowering=False, debug=False,
                   enable_asserts=False, num_devices=B)
    x_d = nc.dram_tensor("x", [H, W, CI], mmdt, kind="ExternalInput").ap()
    g_d = nc.dram_tensor("g", [H, KH * H], mmdt, kind="ExternalInput").ap()
    w_d = nc.dram_tensor("wt", [128, 13 * CO], mmdt,
                         kind="ExternalInput").ap()
    b_d = nc.dram_tensor("bias", [CO, 1], f32, kind="ExternalInput").ap()
    o_d = nc.dram_tensor("out", [2 * NCH, CO, 512], f32,
                         kind="ExternalOutput").ap()

    with tile.TileContext(nc) as tc:
        with (
            tc.tile_pool(name="const", bufs=1) as cpool,
            tc.tile_pool(name="u", bufs=1) as upool,
            tc.tile_pool(name="oacc", bufs=3) as opool,
            tc.tile_pool(name="ps1", bufs=3, space="PSUM") as ps1,
            tc.tile_pool(name="ps2", bufs=2, space="PSUM") as ps2,
        ):
            # DMA descriptors drain FIFO per engine; order by first use:
            # gt + x chunk 7 gate the first matmul, wt only the first
            # stage-2 chunk (~12 tiles in), later x chunks stream behind.
            gt = cpool.tile([H, KH * H], mmdt)
            nc.sync.dma_start(gt[:], g_d)
            xT = cpool.tile([H, W * 2 * CI], mmdt)
            x4 = xT[:].rearrange("p (w di) -> p w di", di=2 * CI)

            def load_x_span(w0, w1):
                # HBM load of the 64 real channels, then duplicate on-chip
                # (DVE SBUF->SBUF bf16 runs in 4x mode) so the stage-1
                # stationary spans all 128 PE columns.
                sl = slice(w0, w1)
                nc.sync.dma_start(x4[:, sl, 0:CI], x_d[:, sl, :])
                nc.vector.tensor_copy(x4[:, sl, CI:2 * CI], x4[:, sl, 0:CI])

            load_x_span(112, 128)
            load_x_span(0, 16)
            wt = cpool.tile([128, 13 * CO], mmdt)
            nc.sync.dma_start(wt[:], w_d)
            bt = cpool.tile([CO, 1], f32)
            nc.sync.dma_start(bt[:], b_d)
            for wc in [1, 2, 3, 4, 5, 6]:
                load_x_span(wc * 16, (wc + 1) * 16)

            import concourse.mybir as _mb

            # GPSIMD cannot read PSUM (BIR verifier) -> DVE + Act only
            cast_engines = [nc.vector.tensor_copy, nc.scalar.copy]
            state = {"cast_idx": 0}

            def next_eng():
                e = cast_engines[state["cast_idx"] % 2]
                state["cast_idx"] += 1
                return e

            def s1_tile(mh, jp, u4):
                p1 = ps1.tile([128, 1024], f32)
                for dj in range(2):
                    wg = (2 * jp + dj - HALO) % W
                    nc.tensor.matmul(p1[:, dj * 512:dj * 512 + KH * MH],
                                     x4[:, wg, :],
                                     gt[:, mh * KH * MH:(mh + 1) * KH * MH],
                                     start=True, stop=True)
                pv = p1[:].rearrange("p (j s m) -> p j s m", j=2, s=8)
                # psum s-order [0,2,4,1,3]: half0 cols 0:192, half1 192:320
                next_eng()(
                    u4[0:64, :, 2 * jp:2 * jp + 2, :].transpose([0, 2, 1, 3]),
                    pv[0:64, :, 0:3, :])
                next_eng()(
                    u4[64:128, 0:2, 2 * jp:2 * jp + 2, :].transpose([0, 2, 1, 3]),
                    pv[64:128, :, 3:5, :])

            def s4_shift_piece(p, u4):
                # us4sh[j] = u_s4[j-1] on partitions 64:127 (c=2 slot)
                j0, j1 = (1, 8) if p == 0 else (8 * p, 8 * p + 8)
                j1 = min(j1, JW)
                nc.sync.dma_start(u4[64:128, 2, j0:j1, :],
                                  u4[0:64, 2, j0 - 1:j1 - 1, :])

            def s2_chunk(mh, ch, u4):
                p2 = ps2.tile([128, 512], f32)
                for gi, (t, c) in enumerate(G10):
                    rhs = u4[0:128, c, ch * 8 + HALO - t:ch * 8 + HALO - t + 8, :]
                    nc.tensor.matmul(p2[:], wt[:, gi * CO:(gi + 1) * CO], rhs,
                                     start=(gi == 0), stop=False)
                for p in range(3):
                    tp = 2 * p
                    rhs = u4[0:128, 2,
                             ch * 8 + HALO - tp:ch * 8 + HALO - tp + 8, :]
                    nc.tensor.matmul(p2[:], wt[:, (10 + p) * CO:(11 + p) * CO],
                                     rhs, start=False, stop=(p == 2))
                oa = opool.tile([CO, 512], f32)
                nc.scalar.activation(oa[:], p2[:],
                                     _mb.ActivationFunctionType.Identity,
                                     bias=bt[:])
                nc.sync.dma_start(o_d[mh * NCH + ch], oa[:])

            NT = JW // 2          # 66 stage-1 tiles (2 w-columns each) per mh
            NPIECE = NCH + 1      # 17 shift pieces per mh
            u4s = {}
            pieces_done = {0: 0, 1: 0}

            def emit_tile(gti):
                mh, jp = divmod(gti, NT)
                if mh not in u4s:
                    u = upool.tile([128, 3 * JW * MH], mmdt, tag=f"u{mh}")
                    u4s[mh] = u[:].rearrange("p (c j m) -> p c j m",
                                             c=3, j=JW)
                    nc.vector.memset(u4s[mh][64:128, 2, 0:1, :], 0.0)
                s1_tile(mh, jp, u4s[mh])
                # shift piece p is ready once tiles jp<=4p+3 are cast
                while pieces_done[mh] < NPIECE and \
                        jp + 1 >= min(4 * pieces_done[mh] + 4, NT):
                    s4_shift_piece(pieces_done[mh], u4s[mh])
                    pieces_done[mh] += 1

            ti = 0
            for gch in range(2 * NCH):
                mh, k = divmod(gch, NCH)
                target = min(mh * NT + min(NT, 4 * k + 8) + LA, 2 * NT)
                while ti < target:
                    emit_tile(ti)
                    ti += 1
                s2_chunk(mh, k, u4s[mh])
            while ti < 2 * NT:
                emit_tile(ti)
                ti += 1
    nc.compile()
    return nc


def _get_prog():
    global _PROG
    if _PROG is None:
        _PROG = _build_program()
    return _PROG


def _build_wstack(weight):
    # wst[(d,i), (g, o)]: 13 groups; g<10: rows 0:64 s=2c, 64:128 s=2c+1;
    # g=10+p: rows 0:64 (s=4, t=2p), rows 64:128 (s=4, t=2p+1) (zeros p=2)
    wst = np.zeros((128, 13 * CO), np.float32)
    for gi, (t, c) in enumerate(G10):
        col = gi * CO
        wst[0:64, col:col + CO] = weight[:, :, 2 * c, t].T
        wst[64:128, col:col + CO] = weight[:, :, 2 * c + 1, t].T
    for p in range(3):
        col = (10 + p) * CO
        tp = 2 * p
        wst[0:64, col:col + CO] = weight[:, :, 4, tp].T
        if tp + 1 < KW:
            wst[64:128, col:col + CO] = weight[:, :, 4, tp + 1].T
    return np.ascontiguousarray(wst).astype(_np_dt())


def kernel(x, weight, bias):
    from concourse.bass_utils import run_bass_kernel_spmd

    global _CONSTS
    if _CONSTS is None:
        _CONSTS = _build_consts()
    GT = _CONSTS

    x = np.ascontiguousarray(np.asarray(x, dtype=np.float32))
    weight = np.ascontiguousarray(np.asarray(weight, dtype=np.float32))
    bias = np.ascontiguousarray(np.asarray(bias, dtype=np.float32))

    wst = _build_wstack(weight)
    b2 = np.ascontiguousarray(bias.reshape(CO, 1))

    in_maps = []
    for b in range(B):
        xh = np.ascontiguousarray(x[b].transpose(1, 2, 0))   # [h, w, i]
        xh = xh.reshape(H, W * CI).astype(_np_dt())
        in_maps.append({"x": xh, "g": GT, "wt": wst, "bias": b2})

    res = run_bass_kernel_spmd(_get_prog(), in_maps, core_ids=list(range(B)),
                               **_RUN_OPTS)
    global _LAST_RESULT
    _LAST_RESULT = res
    outs = []
    for b in range(B):
        arr = res.results[b]["out"].reshape(2, NCH, CO, 8, MH)
        outs.append(arr.transpose(2, 0, 4, 1, 3).reshape(CO, H, W))
    out = np.stack(outs, axis=0)
    return np.ascontiguousarray(out.astype(np.float32))
